# revision 19
# baseline (speedup 1.0000x reference)
"""Shapelet distance transform kernel for Trainium2 (8 NeuronCores).

out[b, s] = min_w sum_{l,c} (data[b, w+l, c] - kernel[s, l, c])^2 / LS

Strategy (data-parallel over batch, 4 batches per core, no collectives):
  dist[s, w] = (a2[w] + k2[s] - 2*cross[s, w]) / LS

v4 design ("channel-plane im2col, TT-chain drain"):
  - Channels padded 3 -> 4 with the 4th channel carrying d2[t] = sum_c
    data[t, c]^2, so the sliding-norm taps fold into one 128-deep
    contraction (kaug rows c<3 = -2/LS * K, c==3 rows = 1/LS ->
    PSUM = (a2 - 2*cross)/LS directly).
  - Data staged to DRAM as CHANNEL PLANES flatpT[c, t] (bf16).  An im2col
    rhs tile X[(c,l), w] = flatpT[c, w0 + l + w] is then 128 *contiguous*
    per-partition lines -> plain async DMA at full line rate.  No PE
    transposes, no xbar DMA transpose, no ACT assembly.
  - bf16 matmuls kaug[128,128]^T @ X[128,512]: one N=512 MM per (group, sc).
  - Min-drain of PSUM split across ACT and DVE (tensor_tensor_reduce is
    broken on this HW -- exec-unit crash -- and tensor_scalar+accum /
    tensor_reduce / Max8 all run at 1x):
      * DIRECT tiles: DVE tensor_reduce(min) straight from PSUM (1x).
      * STAGED tiles: ACT copies PSUM -> SBUF bf16 (1 elem/cyc @1.2GHz),
        then DVE chains them with TENSOR_TENSOR min in true 2x mode
        (measured 2282ns / 4096 elem).
  - + k2[s]/LS, then DMA out.
"""

import sys

for _p in ("/opt/trn_rl_repo",):
    if _p not in sys.path:
        sys.path.insert(0, _p)

from contextlib import ExitStack

import ml_dtypes
import numpy as np

import concourse.bacc as bacc
import concourse.bass as bass
import concourse.tile as tile
from concourse import mybir

F32 = mybir.dt.float32
BF16 = mybir.dt.bfloat16
MIN = mybir.AluOpType.min

B, T, C = 32, 8192, 3
NS, LS = 256, 32
W = T - LS + 1  # 8161 valid windows
NCORES = 8
BL = B // NCORES  # 4 batches per core
FLAT = T * C  # 24576
TPLANE = T + LS  # one channel plane, padded so window reads stay in bounds
TP = 4 * TPLANE  # padded flat length per batch (4 planes)
SCALE = 1.0 / LS
GW = 512  # windows per group (one PSUM bank)
# Streamed windows cover 0..T-1; invalid tails (w > W-1) read the pad:
# data planes pad = 0, d2 plane pad = 1e4 -> dist >= ~190 >> any real min.
PADV = 1.0e4
# Per (batch, sc): 8 two-bank dist tiles; True -> ACT-staged + DVE TT-chain,
# False -> DVE direct tensor_reduce.  6/2 split balances ACT (~1.0us/copy)
# vs DVE (1.19us/direct, 0.69us/chain-link, 1.19us/final).
STAGED = [True, True, False, True, True, False, True, True]


def build_program() -> bass.Bass:
    nc = bacc.Bacc("TRN2", target_bir_lowering=False, debug=False)
    data = nc.dram_tensor("data", [BL, FLAT], BF16, kind="ExternalInput").ap()
    kaug = nc.dram_tensor("kaug", [128, 2, 128], BF16, kind="ExternalInput").ap()
    pre = nc.dram_tensor("pre", [128, 192], F32, kind="ExternalInput").ap()
    padh = nc.dram_tensor("padh", [4, LS], BF16, kind="ExternalInput").ap()
    out = nc.dram_tensor("out", [BL, NS], F32, kind="ExternalOutput").ap()
    flatpT = nc.dram_tensor("flatpT", [BL, 4, TPLANE], BF16).ap()

    with tile.TileContext(nc) as tc, ExitStack() as ctx:
        consts = ctx.enter_context(tc.tile_pool(name="consts", bufs=1))
        kaug_sb = consts.tile([128, 2, 128], BF16)
        nc.sync.dma_start(out=kaug_sb, in_=kaug)
        pre_sb = consts.tile([128, 192], F32)
        nc.sync.dma_start(out=pre_sb, in_=pre)
        k2c0 = consts.tile([128, 1], F32)
        k2c1 = consts.tile([128, 1], F32)
        k2c = [k2c0, k2c1]
        padc = consts.tile([4, LS], BF16)
        nc.sync.dma_start(out=padc, in_=padh)

        # ---- Phase A: write channel planes flatpT[c, t] (+ d2 plane). ----
        prep = ctx.enter_context(tc.tile_pool(name="prep", bufs=4))
        for b in range(BL):
            dt_ = prep.tile([128, 192], BF16, tag="dt")
            nc.scalar.dma_start(out=dt_, in_=data[b].rearrange("(p f) -> p f", p=128))
            sq = prep.tile([128, 192], F32, tag="sq")
            nc.scalar.square(sq, dt_)
            d2t = prep.tile([128, 64, 1], F32, tag="d2t")
            nc.vector.tensor_reduce(
                d2t,
                sq.rearrange("p (t c) -> p t c", c=3),
                axis=mybir.AxisListType.X,
                op=mybir.AluOpType.add,
            )
            # plane-ordered staging tile: pt[p, c, t'] with c==3 -> d2
            pt = prep.tile([128, 4, 64], BF16, tag="pt")
            nc.scalar.copy(pt[:, 0:3, :], dt_.rearrange("p (t c) -> p c t", c=3))
            with nc.allow_low_precision(reason="bf16 staging of d2 taps"):
                nc.vector.tensor_copy(
                    pt[:, 3, :], d2t.rearrange("p t o -> p (t o)")
                )
            # partition p holds t=64p..64p+63; write (p, c) lines of 64
            # contiguous elements at plane offset c*TPLANE + 64p
            nc.scalar.dma_start(
                out=bass.AP(
                    tensor=flatpT.tensor,
                    offset=flatpT.offset + b * TP,
                    ap=[[64, 128], [TPLANE, 4], [1, 64]],
                ),
                in_=pt,
            )
            # pad tails of each plane (data 0, d2 +PADV)
            nc.scalar.dma_start(
                out=bass.AP(
                    tensor=flatpT.tensor,
                    offset=flatpT.offset + b * TP + T,
                    ap=[[TPLANE, 4], [1, LS]],
                ),
                in_=padc,
            )

        ksq = prep.tile([128, 192], F32, tag="ksq")
        nc.scalar.square(ksq, pre_sb)
        for sc in range(2):
            k2raw = prep.tile([128, 1], F32, tag="k2raw")
            nc.vector.tensor_reduce(
                k2raw,
                ksq[:, sc * 96 : (sc + 1) * 96],
                axis=mybir.AxisListType.X,
                op=mybir.AluOpType.add,
            )
            nc.vector.tensor_scalar_mul(k2c[sc], k2raw, SCALE)

        # ---- Phase B: main loop. ----
        x_pool = ctx.enter_context(tc.tile_pool(name="xp", bufs=2))
        stg_pool = ctx.enter_context(tc.tile_pool(name="stg", bufs=3))
        acc_pool = ctx.enter_context(tc.tile_pool(name="accp", bufs=2))
        part_pool = ctx.enter_context(tc.tile_pool(name="part", bufs=2))
        fin_pool = ctx.enter_context(tc.tile_pool(name="fin", bufs=4))
        ps = ctx.enter_context(tc.tile_pool(name="ps", bufs=1, space="PSUM"))

        for b in range(BL):
            # X[(32c+l), (g, j)] = flatpT[b, c, l + 512 g + j]: whole batch,
            # one DMA per channel plane (32 lines of 16 KB contiguous each)
            xg = x_pool.tile([128, 16, GW], BF16, tag="x")
            for half in range(2):
                for c in range(4):
                    nc.sync.dma_start(
                        out=xg[32 * c : 32 * (c + 1), 8 * half : 8 * (half + 1), :],
                        in_=bass.AP(
                            tensor=flatpT.tensor,
                            offset=flatpT.offset + b * TP + c * TPLANE + half * (T // 2),
                            ap=[[1, 32], [1, T // 2]],
                        ),
                    )
            m0 = part_pool.tile([128, 3], F32, tag="m0")
            m1 = part_pool.tile([128, 3], F32, tag="m1")
            mins = [m0, m1]
            accs = [None, None]
            nst = [0, 0]
            prev_stg = [None, None]
            di = [0, 0]
            for t in range(8):
                rhs = [xg[:, 2 * t, :], xg[:, 2 * t + 1, :]]
                dt0 = ps.tile([128, 2, GW], F32, tag="P0", bufs=2)
                dt1 = ps.tile([128, 2, GW], F32, tag="P1", bufs=2)
                dtl = [dt0, dt1]
                for j in range(2):
                    for sc in range(2):
                        nc.tensor.matmul(
                            dtl[sc][:, j, :],
                            kaug_sb[:, sc, :],
                            rhs[j],
                            start=True,
                            stop=True,
                        )
                for sc in range(2):
                    with nc.allow_low_precision(reason="bf16 min chain"):
                        if STAGED[t]:
                            stg = stg_pool.tile([128, 2 * GW], BF16, tag=f"s{sc}")
                            nc.scalar.copy(
                                stg, dtl[sc].rearrange("p a b -> p (a b)")
                            )
                            nst[sc] += 1
                            if nst[sc] == 1:
                                prev_stg[sc] = stg
                            elif nst[sc] == 2:
                                acc = acc_pool.tile(
                                    [128, 2 * GW], BF16, tag=f"a{sc}"
                                )
                                nc.vector.tensor_tensor(
                                    out=acc, in0=prev_stg[sc], in1=stg, op=MIN
                                )
                                accs[sc] = acc
                            else:
                                acc2 = acc_pool.tile(
                                    [128, 2 * GW], BF16, tag=f"a{sc}"
                                )
                                nc.vector.tensor_tensor(
                                    out=acc2, in0=accs[sc], in1=stg, op=MIN
                                )
                                accs[sc] = acc2
                        else:
                            nc.vector.tensor_reduce(
                                mins[sc][:, di[sc] : di[sc] + 1],
                                dtl[sc],
                                axis=mybir.AxisListType.XY,
                                op=MIN,
                            )
                            di[sc] += 1
            for sc in range(2):
                # fold the staged-chain acc into the partials, then finish
                nc.vector.tensor_reduce(
                    mins[sc][:, di[sc] : di[sc] + 1],
                    accs[sc],
                    axis=mybir.AxisListType.X,
                    op=MIN,
                )
                res = fin_pool.tile([128, 1], F32, tag="res")
                nc.vector.tensor_reduce(
                    res,
                    mins[sc][:, 0 : di[sc] + 1],
                    axis=mybir.AxisListType.X,
                    op=MIN,
                )
                fin = fin_pool.tile([128, 1], F32, tag="fin")
                nc.vector.tensor_scalar(
                    out=fin,
                    in0=res,
                    scalar1=k2c[sc],
                    scalar2=None,
                    op0=mybir.AluOpType.add,
                )
                nc.sync.dma_start(
                    out=out[b, sc * 128 : (sc + 1) * 128].rearrange(
                        "(p o) -> p o", o=1
                    ),
                    in_=fin,
                )
    nc.compile()
    return nc


_PROGRAM = None


def _get_program() -> bass.Bass:
    global _PROGRAM
    if _PROGRAM is None:
        _PROGRAM = build_program()
    return _PROGRAM


def make_in_maps(data: np.ndarray, kernel: np.ndarray) -> list[dict]:
    assert data.shape == (B, T, C) and kernel.shape == (NS, LS, C)
    flat = (
        np.ascontiguousarray(data, dtype=np.float32)
        .reshape(B, FLAT)
        .astype(ml_dtypes.bfloat16)
    )
    # round the kernel through bf16 so k2 is consistent with the bf16 kaug
    # actually used in the matmul
    kb = (
        np.ascontiguousarray(kernel, dtype=np.float32)
        .astype(ml_dtypes.bfloat16)
        .astype(np.float32)
    )
    pre = np.empty((128, 192), dtype=np.float32)
    pre[:, 0:96] = kb[0:128].reshape(128, 96)
    pre[:, 96:192] = kb[128:256].reshape(128, 96)
    # kaug row order matches X partition order p = 32*c + l
    ka = np.zeros((128, 2, 128), dtype=np.float32)
    for sc in range(2):
        kk = kb[sc * 128 : (sc + 1) * 128]  # [128 s, 32 l, 3 c]
        a = np.zeros((4, 32, 128), dtype=np.float32)
        a[0:3, :, :] = (-2.0 * SCALE) * kk.transpose(2, 1, 0)
        a[3, :, :] = SCALE
        ka[:, sc, :] = a.reshape(128, 128)
    kaug = ka.astype(ml_dtypes.bfloat16)
    padh = np.zeros((4, LS), dtype=np.float32)
    padh[3, :] = PADV
    padh = padh.astype(ml_dtypes.bfloat16)
    return [
        {
            "data": np.ascontiguousarray(flat[i * BL : (i + 1) * BL]),
            "kaug": kaug,
            "pre": pre,
            "padh": padh,
        }
        for i in range(NCORES)
    ]


def kernel(data: np.ndarray, kernel: np.ndarray) -> np.ndarray:
    from concourse.bass_utils import run_bass_kernel_spmd

    in_maps = make_in_maps(data, kernel)
    nc = _get_program()
    res = run_bass_kernel_spmd(nc, in_maps, list(range(NCORES)))
    return np.concatenate(
        [res.results[i]["out"] for i in range(NCORES)], axis=0
    ).astype(np.float32)
